# revision 1
# baseline (speedup 1.0000x reference)
"""Trainium2 Bass kernel for the quantized LM-head (nn_LmHeadTender).

Math (per core, vocab-sharded):
  reference computes
    Wl   = dequant_int4(lm_weight)            # per-row scale sw = rowmax/7
    y    = dequant_int4(x, per-(chunk,channel) scale s = tmax*2^(bucket-13)/7)
    out  = y @ Wl.T
  We factor every scale out of the matmul so that both matmul operands are
  small integers (times powers of two) that are EXACTLY representable in
  bf16:
    qw  in [-8, 7]                 (weight int values)
    yq  = qx * 2^(bucket-13)       (activation ints scaled by a power of 2)
    out[t, v] = (tmax_c/7) * sw[v] * sum_h yq[t, h] * qw[v, h]
  The bf16 matmul therefore computes exact products accumulated in fp32
  PSUM - the result matches the f32 reference to ~1e-6 (accumulation
  order), at bf16 matmul speed.

Sharding: lm_weight split into 8 vocab shards of 4000 rows, zero-padded to
4096.  hidden_states replicated.  Host concatenates the per-core [4096,
4096] logits (first 4000 cols valid) along vocab.
"""

import os
import sys
from contextlib import ExitStack

import numpy as np

import concourse.bass as bass
import concourse.tile as tile
from concourse import bacc, masks, mybir
from concourse.bass_utils import run_bass_kernel_spmd

FP = mybir.dt.float32
BF = mybir.dt.bfloat16
I32 = mybir.dt.int32
ALU = mybir.AluOpType
AX = mybir.AxisListType

T = 4096            # tokens (2*2048)
H = 4096            # hidden
V = 32000           # vocab
NCORE = 8
VSH = V // NCORE    # 4000 valid vocab rows per core
VP = 4096           # padded per-core vocab
CHUNK = 256
NCHUNK = T // CHUNK  # 16
DECOMP = 14
QMAX = 7.0
C_MAGIC = 12582912.0   # 1.5 * 2^23: round-to-nearest-even via add/sub
C7 = float(np.float32(1.0) / np.float32(7.0))  # fl(1/7); DVE has no divide op

KT = H // 128       # 32 k tiles
MT = VP // 128      # 32 weight row tiles
NT_GROUPS = 2       # token groups for the matmul phase
TG = T // (NT_GROUPS * 128)   # 16 token tiles (of 128) per group
VB = VP // 512      # 8 vocab blocks of 512


def _emit(ctx: ExitStack, tc: "tile.TileContext", x_d, w_d, out_d):
    nc = tc.nc

    # ---------------- persistent tiles ----------------
    cpool = ctx.enter_context(tc.tile_pool(name="consts", bufs=1))
    ident = cpool.tile([128, 128], FP)
    masks.make_identity(nc, ident[:])
    ones_row = cpool.tile([1, 128], FP)
    nc.vector.memset(ones_row[:], 1.0)
    sw_pk = cpool.tile([128, 32], FP)      # sw packed [p, m]; v = m*128+p
    sw_t = cpool.tile([32, 128], FP)       # sw transposed [m, p]
    sw_row = cpool.tile([1, VP], FP)       # sw on one partition, v-major
    sw_rep = cpool.tile([128, VP], FP)     # sw replicated on all partitions
    m7_all = cpool.tile([128, 16], FP)     # tmax_c/7 broadcast, col per chunk

    dpool = ctx.enter_context(tc.tile_pool(name="dram", bufs=1, space="DRAM"))
    qw_d = dpool.tile([VP, H], BF)         # quantized weight ints, [v, h]
    y_d = dpool.tile([H, T], BF)           # quantized act * 2^(b-13), [h, t]
    sw_d = dpool.tile([32, 128], FP)       # sw bounce buffer (row-major = v)

    # ---------------- weight phase ----------------
    with (
        tc.tile_pool(name="wq", bufs=2) as wq_pool,
        tc.tile_pool(name="wsm", bufs=2) as ws_pool,
    ):
        for m in range(MT):
            w_nat = wq_pool.tile([128, H], FP, tag="w_nat")
            nc.sync.dma_start(w_nat[:], w_d[m * 128:(m + 1) * 128, :])
            rmax = ws_pool.tile([128, 1], FP, tag="rmax")
            nc.vector.tensor_reduce(
                rmax[:], w_nat[:], axis=AX.X, op=ALU.max,
                apply_absolute_value=True)
            # sw = max(rmax*(1/7), 1e-9)  (reference: max(rmax/7, 1e-9))
            nc.vector.tensor_scalar(
                sw_pk[:, m:m + 1], rmax[:], C7, 1e-9, ALU.mult, ALU.max)
            rw = ws_pool.tile([128, 1], FP, tag="rw")
            nc.vector.reciprocal(rw[:], sw_pk[:, m:m + 1])
            # quantize in place: round(w*rw) clamped to [-8, 7]
            nc.vector.tensor_scalar(
                w_nat[:], w_nat[:], rw[:], C_MAGIC, ALU.mult, ALU.add)
            nc.vector.tensor_scalar(
                w_nat[:], w_nat[:], C_MAGIC, QMAX, ALU.subtract, ALU.min)
            qw_st = wq_pool.tile([128, H], BF, tag="qw_st")
            nc.vector.tensor_scalar(
                qw_st[:], w_nat[:], -(QMAX + 1.0), None, ALU.max)
            nc.sync.dma_start(qw_d[m * 128:(m + 1) * 128, :], qw_st[:])

    # ---------------- sw_rep build ----------------
    with tc.tile_pool(name="swps", bufs=4, space="PSUM") as swps_pool:
        for a in range(4):
            nc.vector.transpose(
                sw_t[:, a * 32:(a + 1) * 32], sw_pk[a * 32:(a + 1) * 32, :])
        nc.sync.dma_start(sw_d[:, :], sw_t[:])
        nc.sync.dma_start(sw_row[:], sw_d[:, :])
        for j in range(VP // 512):
            bp = swps_pool.tile([128, 512], FP, tag="bp")
            nc.tensor.matmul(
                bp[:], ones_row[:], sw_row[:, j * 512:(j + 1) * 512],
                start=True, stop=True)
            nc.scalar.copy(sw_rep[:, j * 512:(j + 1) * 512], bp[:])

    # ---------------- activation phase ----------------
    with (
        tc.tile_pool(name="xin", bufs=3) as xin_pool,
        tc.tile_pool(name="xT", bufs=2) as xT_pool,
        tc.tile_pool(name="xst", bufs=2) as st_pool,
        tc.tile_pool(name="yst", bufs=8) as y_pool,
        tc.tile_pool(name="xps", bufs=6, space="PSUM") as xps_pool,
        tc.tile_pool(name="bps", bufs=2, space="PSUM") as bps_pool,
    ):
        for c in range(NCHUNK):
            xT = xT_pool.tile([128, KT, CHUNK], FP, tag="xT")
            for th in range(2):
                xnat = xin_pool.tile([128, H], FP, tag="xn")
                nc.sync.dma_start(
                    xnat[:],
                    x_d[c * CHUNK + th * 128: c * CHUNK + (th + 1) * 128, :])
                for i in range(KT):
                    pst = xps_pool.tile([128, 128], FP, tag="pst")
                    nc.tensor.transpose(
                        pst[:], xnat[:, i * 128:(i + 1) * 128], ident[:])
                    dst = xT[:, i, th * 128:(th + 1) * 128]
                    if (i + th) % 2 == 0:
                        nc.scalar.copy(dst, pst[:])
                    else:
                        nc.vector.tensor_copy(dst, pst[:])
            # ---- stats: cmax per channel, tmax per chunk ----
            cmax = st_pool.tile([128, KT], FP, tag="cmax")
            nc.vector.tensor_reduce(
                cmax[:], xT[:], axis=AX.X, op=ALU.max,
                apply_absolute_value=True)
            tpad = st_pool.tile([128, 32], FP, tag="tpad")
            nc.vector.memset(tpad[:], 0.0)
            nc.vector.tensor_reduce(
                tpad[:, 0:1], cmax[:], axis=AX.X, op=ALU.max)
            tt = st_pool.tile([32, 128], FP, tag="tt")
            for a in range(4):
                nc.vector.transpose(
                    tt[:, a * 32:(a + 1) * 32], tpad[a * 32:(a + 1) * 32, :])
            tmax_sc = st_pool.tile([1, 1], FP, tag="tmax_sc")
            nc.vector.tensor_reduce(
                tmax_sc[:], tt[0:1, :], axis=AX.X, op=ALU.max)
            # broadcast tmax to 128 partitions via PE outer product
            bp1 = bps_pool.tile([128, 1], FP, tag="bp1")
            nc.tensor.matmul(
                bp1[:], ones_row[:], tmax_sc[:], start=True, stop=True)
            tmax_b = st_pool.tile([128, 1], FP, tag="tmax_b")
            nc.scalar.copy(tmax_b[:], bp1[:])
            nc.vector.tensor_scalar(
                m7_all[:, c:c + 1], tmax_b[:], C7, None, ALU.mult)
            # ---- bucket: number of thresholds strictly exceeded ----
            bucket = st_pool.tile([128, KT], FP, tag="bucket")
            nc.vector.memset(bucket[:], 0.0)
            for lv in range(DECOMP - 1):
                thr = st_pool.tile([128, 1], FP, tag="thr", bufs=2)
                nc.vector.tensor_scalar(
                    thr[:], tmax_b[:], 2.0 ** (lv - (DECOMP - 1)), None,
                    ALU.mult)
                nc.vector.scalar_tensor_tensor(
                    bucket[:], cmax[:], thr[:], bucket[:],
                    op0=ALU.is_gt, op1=ALU.add)
            # ---- pw = 2^(bucket-13) exactly, via IEEE bit construction ----
            g = st_pool.tile([128, KT], FP, tag="g")
            nc.vector.tensor_scalar(
                g[:], bucket[:], 114.0, 8388608.0, ALU.add, ALU.mult)
            g_i = st_pool.tile([128, KT], I32, tag="g_i")
            nc.vector.tensor_copy(g_i[:], g[:])
            pw = g_i[:].bitcast(FP)
            # ---- scales: s = max(tmax*pw/7, 1e-9); r = 1/s ----
            ch_thr = st_pool.tile([128, KT], FP, tag="ch_thr")
            nc.vector.tensor_scalar(
                ch_thr[:], pw, tmax_b[:], None, ALU.mult)
            s_t = st_pool.tile([128, KT], FP, tag="s_t")
            nc.vector.tensor_scalar(
                s_t[:], ch_thr[:], C7, 1e-9, ALU.mult, ALU.max)
            r_t = st_pool.tile([128, KT], FP, tag="r_t")
            nc.vector.reciprocal(r_t[:], s_t[:])
            # ---- quantize: y = clip(round(x*r), -8, 7) * pw  (bf16) ----
            for i in range(KT):
                sl = xT[:, i, :]
                nc.vector.tensor_scalar(
                    sl, sl, r_t[:, i:i + 1], C_MAGIC, ALU.mult, ALU.add)
                nc.vector.tensor_scalar(
                    sl, sl, C_MAGIC, QMAX, ALU.subtract, ALU.min)
                y_st = y_pool.tile([128, CHUNK], BF, tag="y_st")
                pw_col = g_i[:, i:i + 1].bitcast(FP)
                nc.vector.tensor_scalar(
                    y_st[:], sl, -(QMAX + 1.0), pw_col, ALU.max, ALU.mult)
                nc.sync.dma_start(
                    y_d[i * 128:(i + 1) * 128, c * CHUNK:(c + 1) * CHUNK],
                    y_st[:])

    # ---------------- matmul phase ----------------
    with (
        tc.tile_pool(name="ymm", bufs=1) as ymm_pool,
        tc.tile_pool(name="qwp", bufs=KT + 2) as qw_pool,
        tc.tile_pool(name="stg", bufs=4) as stg_pool,
        tc.tile_pool(name="mps", bufs=8, space="PSUM") as mps_pool,
    ):
        tok_g = TG * 128  # tokens per group
        for grp in range(NT_GROUPS):
            y_all = ymm_pool.tile([128, KT, tok_g], BF, tag="y_all")
            for k in range(KT):
                nc.sync.dma_start(
                    y_all[:, k, :],
                    y_d[k * 128:(k + 1) * 128,
                        grp * tok_g:(grp + 1) * tok_g])
            for vb in range(VB):
                qwt = []
                for k in range(KT):
                    qt = qw_pool.tile([128, 512], BF, tag="qw")
                    nc.sync.dma_start(
                        qt[:],
                        qw_d[vb * 512:(vb + 1) * 512,
                             k * 128:(k + 1) * 128],
                        transpose=True)
                    qwt.append(qt)
                for t in range(TG):
                    ps = mps_pool.tile([128, 512], FP, tag="ps")
                    for k in range(KT):
                        nc.tensor.matmul(
                            ps[:],
                            y_all[:, k, t * 128:(t + 1) * 128],
                            qwt[k][:],
                            start=(k == 0), stop=(k == KT - 1))
                    stg = stg_pool.tile([128, 512], FP, tag="stg")
                    tidx = grp * TG + t
                    cch = (tidx * 128) // CHUNK
                    nc.vector.scalar_tensor_tensor(
                        stg[:], ps[:], m7_all[:, cch:cch + 1],
                        sw_rep[:, vb * 512:(vb + 1) * 512],
                        op0=ALU.mult, op1=ALU.mult)
                    nc.sync.dma_start(
                        out_d[tidx * 128:(tidx + 1) * 128,
                              vb * 512:(vb + 1) * 512],
                        stg[:])


_CACHED = None


def _build():
    global _CACHED
    if _CACHED is not None:
        return _CACHED
    nc = bacc.Bacc(
        "TRN2", target_bir_lowering=False, debug=False,
        enable_asserts=False, num_devices=NCORE)
    x_d = nc.dram_tensor("x", (T, H), FP, kind="ExternalInput").ap()
    w_d = nc.dram_tensor("w", (VP, H), FP, kind="ExternalInput").ap()
    out_d = nc.dram_tensor("out", (T, VP), FP, kind="ExternalOutput").ap()
    with tile.TileContext(nc) as tc:
        with ExitStack() as ctx:
            _emit(ctx, tc, x_d, w_d, out_d)
    nc.compile()
    _CACHED = nc
    return nc


def kernel(hidden_states: np.ndarray, lm_weight: np.ndarray) -> np.ndarray:
    b, t, h = hidden_states.shape
    assert (b * t, h) == (T, H) and lm_weight.shape == (V, H)
    x_full = np.ascontiguousarray(
        hidden_states.reshape(T, H).astype(np.float32))
    in_maps = []
    for c in range(NCORE):
        shard = np.zeros((VP, H), dtype=np.float32)
        shard[:VSH] = lm_weight[c * VSH:(c + 1) * VSH]
        in_maps.append({"x": x_full, "w": shard})
    nc = _build()
    res = run_bass_kernel_spmd(nc, in_maps, core_ids=list(range(NCORE)))
    outs = [res.results[c]["out"][:, :VSH] for c in range(NCORE)]
    full = np.concatenate(outs, axis=1)
    return full.reshape(b, t, V)



# revision 11
# speedup vs baseline: 1.7044x; 1.7044x over previous
"""Trainium2 Bass kernel for the quantized LM-head (nn_LmHeadTender), v2.

Math (per core, vocab-sharded; V shard = 4000 rows):
    Wl   = dequant_int4(lm_weight)        # per-row scale sw = rowmax/7
    y    = dequant_int4(x, per-(chunk,channel) scale s = tmax*2^(b-13)/7)
    out  = y @ Wl.T
Factor every scale out of the matmul so both operands are small integers
(times powers of two), exactly representable in fp8 e5m2:
    qw  in [-7, 7]                  (weight ints; |w/s| <= 7 so no clip)
    yq  = qx * 2^(bucket-13)        (activation ints scaled by power of 2)
    out[t, v] = (tmax_c/7) * sw[v] * sum_h yq[t, h] * qw[v, h]
The fp8 DoubleRow matmul (2 k-tiles per pass) runs at 2x bf16 rate with
exact products accumulated in fp32 PSUM.  The rank-1 output scale
m7[chunk] * sw[v] is applied on the host (exact same f32 values the
device divided by), so the device ships raw bf16 logits.

Pipeline per core:
  - weights: DMA [v,h] f32 -> DVE rowmax+quant -> ACT round -> fp8 ->
    PE transpose -> qw resident in SBUF as [h, v] fp8 (125 KB/partition)
  - acts: DMA [t,h] f32 -> gpsimd partition_all_reduce(absmax) for
    per-channel maxes (bounced via DRAM into packed [h%128, h//128]
    layout) -> bucket/scale stats on DVE -> PE transpose x (f32) ->
    ACT drains PSUM with fused  x*r + magic  (per-partition scale) ->
    DVE/ACT  (q-magic)*2^(b-13) -> y fp8 [h, t]
  - matmul: per chunk, 2x8 chains of 16 DoubleRow matmuls vs resident
    qw -> PSUM f32 -> DVE/ACT copy to bf16 -> DMA out.
Weight v-blocks are interleaved with chunk-0 chains so the PE starts
matmuls as soon as the first vocab block of qw is ready.
"""

import numpy as np

from contextlib import ExitStack

import concourse.bass as bass
import concourse.tile as tile
from concourse import bacc, bass_isa, masks, mybir
from concourse.bass_utils import run_bass_kernel_spmd

FP = mybir.dt.float32
BF = mybir.dt.bfloat16
F8 = mybir.dt.float8e5
I32 = mybir.dt.int32
ALU = mybir.AluOpType
AX = mybir.AxisListType
ACTF = mybir.ActivationFunctionType
DR = mybir.MatmulPerfMode.DoubleRow
RED = bass_isa.ReduceOp

T = 4096            # tokens (2*2048)
H = 4096            # hidden
V = 32000           # vocab
NCORE = 8
VSH = V // NCORE    # 4000 vocab rows per core
CHUNK = 256
NCHUNK = T // CHUNK         # 16
KT = H // 128               # 32 k-tiles
NKP = KT // 2               # 16 k-pairs per chain
HH = H // 2                 # 2048 h-half
DECOMP = 14
QMAX = 7.0
C_MAGIC = 12582912.0        # 1.5 * 2^23: round-to-nearest-even via add/sub
C7 = float(np.float32(1.0) / np.float32(7.0))

WT = 32                     # weight v-tiles (last one has 32 rows)
WT_ROWS = [128] * 31 + [VSH - 31 * 128]
VBS = [512] * 7 + [VSH - 7 * 512]   # v-blocks per chain group (last 416)
NVB = len(VBS)


def _emit(ctx: ExitStack, tc: "tile.TileContext", x_d, w_d, out_d):
    nc = tc.nc

    cpool = ctx.enter_context(tc.tile_pool(name="consts", bufs=1))
    ident = cpool.tile([128, 128], FP)
    masks.make_identity(nc, ident[:])
    identb = cpool.tile([128, 128], BF)
    masks.make_identity(nc, identb[:])
    ident8 = cpool.tile([128, 128], F8)
    nc.vector.tensor_copy(ident8[:], identb[:])
    ones_row = cpool.tile([1, 128], FP)
    nc.vector.memset(ones_row[:], 1.0)
    qw_sb = cpool.tile([128, KT, VSH], F8)   # resident quantized weight [h, v]

    dpool = ctx.enter_context(tc.tile_pool(name="dram", bufs=1, space="DRAM"))
    cmaxb_d = dpool.tile([NCHUNK * 2, H], FP)   # absmax bounce rows

    # persistent pools
    wpool = ctx.enter_context(tc.tile_pool(name="wstg", bufs=2))
    w8pool = ctx.enter_context(tc.tile_pool(name="w8", bufs=1))
    wsml = ctx.enter_context(tc.tile_pool(name="wsml", bufs=2))
    xpool = ctx.enter_context(tc.tile_pool(name="xin", bufs=3))
    gpool = ctx.enter_context(tc.tile_pool(name="gpo", bufs=1))
    spool = ctx.enter_context(tc.tile_pool(name="stats", bufs=1))
    q1pool = ctx.enter_context(tc.tile_pool(name="q1", bufs=3))
    ypool = ctx.enter_context(tc.tile_pool(name="y8", bufs=2))
    opool = ctx.enter_context(tc.tile_pool(name="ostg", bufs=2))

    mmps = ctx.enter_context(tc.tile_pool(name="mmps", bufs=3, space="PSUM"))
    xps = ctx.enter_context(tc.tile_pool(name="xps", bufs=4, space="PSUM"))
    wps = ctx.enter_context(tc.tile_pool(name="wps", bufs=1, space="PSUM"))

    y8_of = {}
    stats_of = {}

    # ---------------- weight tile m: quantize + transpose into qw_sb ----
    def emit_w_tile(m):
        rows = WT_ROWS[m]
        v0 = m * 128
        wn = []
        for hh in range(2):
            t_ = wpool.tile([128, HH], FP, tag=f"wn{hh}", name=f"wn{hh}", bufs=(2 if hh == 0 else 1))
            nc.sync.dma_start(
                t_[:rows, :], w_d[v0:v0 + rows, hh * HH:(hh + 1) * HH])
            wn.append(t_)
        r1 = wsml.tile([128, 2], FP, tag="r1", name="r1")
        for hh in range(2):
            nc.vector.tensor_reduce(
                r1[:rows, hh:hh + 1], wn[hh][:rows, :], axis=AX.X, op=ALU.max,
                apply_absolute_value=True)
        rmax = wsml.tile([128, 1], FP, tag="rmax", name="rmax")
        nc.vector.tensor_reduce(
            rmax[:rows, :], r1[:rows, :], axis=AX.X, op=ALU.max)
        sw = wsml.tile([128, 1], FP, tag="sw", name="sw")
        nc.vector.tensor_scalar(
            sw[:rows, :], rmax[:rows, :], C7, 1e-9, ALU.mult, ALU.max)
        rw = wsml.tile([128, 1], FP, tag="rw", name="rw")
        nc.vector.reciprocal(rw[:rows, :], sw[:rows, :])
        w8 = []
        for hh in range(2):
            # q + magic  (round-to-nearest-even)
            nc.vector.tensor_scalar(
                wn[hh][:rows, :], wn[hh][:rows, :], rw[:rows, :], C_MAGIC,
                ALU.mult, ALU.add)
            q8 = w8pool.tile([128, HH], F8, tag=f"w8{hh}", name=f"q8{hh}")
            nc.scalar.activation(
                q8[:rows, :], wn[hh][:rows, :], ACTF.Copy, bias=-C_MAGIC,
                scale=1.0)
            w8.append(q8)
        # transpose 32 k-blocks into qw_sb[:, k, v0:v0+rows]
        # (fp8 transpose writes PSUM with element step 2 -> interleaved tile)
        for q in range(4):
            ps = wps.tile([128, 8, 128, 2], F8, tag="wps", name="wps")
            for j in range(8):
                k = q * 8 + j
                src = w8[k // 16][:rows, (k % 16) * 128:(k % 16 + 1) * 128]
                nc.tensor.transpose(
                    ps[:, j, :rows, 0], src, ident8[:rows, :rows])
            nc.scalar.copy(
                qw_sb[:, q * 8:(q + 1) * 8, v0:v0 + rows],
                ps[:, :, :rows, 0])

    # ---------------- activation stats for chunk c ----------------------
    def emit_act_stats(c):
        xh = [[None, None], [None, None]]
        for rt in range(2):
            for hh in range(2):
                t_ = xpool.tile([128, HH], FP, tag="x", name="x")
                nc.sync.dma_start(
                    t_[:],
                    x_d[c * CHUNK + rt * 128:c * CHUNK + (rt + 1) * 128,
                        hh * HH:(hh + 1) * HH])
                xh[rt][hh] = t_
                for qq in range(2):
                    gpo = gpool.tile([128, HH // 2], FP, tag="gpo", name="gpo")
                    nc.gpsimd.partition_all_reduce(
                        gpo[:], t_[:, qq * (HH // 2):(qq + 1) * (HH // 2)],
                        128, RED.absmax)
                    nc.sync.dma_start(
                        cmaxb_d[2 * c + rt,
                                hh * HH + qq * (HH // 2):
                                hh * HH + (qq + 1) * (HH // 2)],
                        gpo[0:1, :])
        # load back packed [p, k] per row-tile and combine
        cm_a = spool.tile([128, KT], FP, tag="cm_a", name="cm_a", bufs=2)
        cm_b = spool.tile([128, KT], FP, tag="cm_b", name="cm_b", bufs=2)
        nc.sync.dma_start(
            cm_a[:], cmaxb_d[2 * c, :].rearrange("(k p) -> p k", p=128))
        nc.sync.dma_start(
            cm_b[:], cmaxb_d[2 * c + 1, :].rearrange("(k p) -> p k", p=128))
        cmax = spool.tile([128, KT], FP, tag="cmax", name="cmax")
        nc.vector.scalar_tensor_tensor(
            cmax[:], cm_a[:], 1.0, cm_b[:], op0=ALU.mult, op1=ALU.max)
        # tmax: free reduce + 32x32 transposes + reduce + PE broadcast
        tpad = spool.tile([128, 32], FP, tag="tpad", name="tpad")
        nc.vector.memset(tpad[:], 0.0)
        nc.vector.tensor_reduce(tpad[:, 0:1], cmax[:], axis=AX.X, op=ALU.max)
        tt = spool.tile([32, 128], FP, tag="tt", name="tt")
        for a in range(4):
            nc.vector.transpose(
                tt[:, a * 32:(a + 1) * 32], tpad[a * 32:(a + 1) * 32, :])
        tmax_sc = spool.tile([1, 1], FP, tag="tmax_sc", name="tmax_sc")
        nc.vector.tensor_reduce(tmax_sc[:], tt[0:1, :], axis=AX.X, op=ALU.max)
        tmax_b = spool.tile([128, 1], FP, tag="tmax_b", name="tmax_b")
        nc.gpsimd.partition_broadcast(tmax_b[:], tmax_sc[:])
        # bucket = #(cmax > tmax*2^(lv-13)), lv = 0..12
        bucket = spool.tile([128, KT], FP, tag="bucket", name="bucket")
        nc.vector.memset(bucket[:], 0.0)
        for lv in range(DECOMP - 1):
            thr = spool.tile([128, 1], FP, tag="thr", name="thr", bufs=2)
            nc.vector.tensor_scalar(
                thr[:], tmax_b[:], 2.0 ** (lv - (DECOMP - 1)), None, ALU.mult)
            nc.vector.scalar_tensor_tensor(
                bucket[:], cmax[:], thr[:], bucket[:],
                op0=ALU.is_gt, op1=ALU.add)
        # pw = 2^(bucket-13) via IEEE bit construction
        g = spool.tile([128, KT], FP, tag="g", name="g")
        nc.vector.tensor_scalar(
            g[:], bucket[:], 114.0, 8388608.0, ALU.add, ALU.mult)
        g_i = spool.tile([128, KT], I32, tag="g_i", name="g_i", bufs=2)
        nc.vector.tensor_copy(g_i[:], g[:])
        pw = g_i[:].bitcast(FP)
        ch_thr = spool.tile([128, KT], FP, tag="ch_thr", name="ch_thr")
        nc.vector.tensor_scalar(ch_thr[:], pw, tmax_b[:], None, ALU.mult)
        s_t = spool.tile([128, KT], FP, tag="s_t", name="s_t")
        nc.vector.tensor_scalar(
            s_t[:], ch_thr[:], C7, 1e-9, ALU.mult, ALU.max)
        r_t = spool.tile([128, KT], FP, tag="r_t", name="r_t", bufs=2)
        nc.vector.reciprocal(r_t[:], s_t[:])
        bias_k = spool.tile([128, KT], FP, tag="bias_k", name="bias_k", bufs=2)
        nc.vector.tensor_scalar(bias_k[:], pw, -C_MAGIC, None, ALU.mult)
        stats_of[c] = (g_i, r_t, bias_k)
        return xh

    # ---------------- transpose + fused quant for chunk c ---------------
    def emit_act_transposes(c, xh):
        g_i, r_t, bias_k = stats_of[c]
        y8 = ypool.tile([128, KT, CHUNK], F8, tag="y8", name="y8")
        for tb in range(2):
            for kq in range(KT // 4):
                ps = xps.tile([128, 4, 128], FP, tag="xps", name="ps")
                for j in range(4):
                    k = kq * 4 + j
                    src = xh[tb][k // 16][:, (k % 16) * 128:(k % 16 + 1) * 128]
                    nc.tensor.transpose(ps[:, j, :], src, ident[:])
                for j in range(4):
                    k = kq * 4 + j
                    q1 = q1pool.tile([128, 128], FP, tag="q1", name="q1")
                    nc.scalar.activation(
                        q1[:], ps[:, j, :], ACTF.Copy, bias=C_MAGIC,
                        scale=r_t[:, k:k + 1])
                    dst = y8[:, k, tb * 128:(tb + 1) * 128]
                    pw_col = g_i[:, k:k + 1].bitcast(FP)
                    if k % 2 == 0:
                        nc.vector.tensor_scalar(
                            dst, q1[:], C_MAGIC, pw_col,
                            ALU.subtract, ALU.mult)
                    else:
                        nc.scalar.activation(
                            dst, q1[:], ACTF.Identity,
                            bias=bias_k[:, k:k + 1], scale=pw_col)
        y8_of[c] = y8

    # ---------------- matmul chains for chunk c -------------------------
    def emit_chains(c, vbs=None):
        y8 = y8_of[c]
        for tb in range(2):
            for vb in (range(NVB) if vbs is None else vbs):
                wv = VBS[vb]
                ps = mmps.tile([128, 512], FP, tag="mm", name="mmps")
                for kp in range(NKP):
                    nc.tensor.matmul(
                        ps[:, :wv],
                        y8[:, 2 * kp:2 * kp + 2, tb * 128:(tb + 1) * 128],
                        qw_sb[:, 2 * kp:2 * kp + 2, vb * 512:vb * 512 + wv],
                        start=(kp == 0), stop=(kp == NKP - 1), perf_mode=DR)
                stg = opool.tile([128, 512], BF, tag="stg", name="stg")
                if vb % 2 == 0:
                    nc.vector.tensor_copy(stg[:, :wv], ps[:, :wv])
                else:
                    nc.scalar.copy(stg[:, :wv], ps[:, :wv])
                nc.sync.dma_start(
                    out_d[c * CHUNK + tb * 128:c * CHUNK + (tb + 1) * 128,
                          vb * 512:vb * 512 + wv],
                    stg[:, :wv])

    # ---------------- emission schedule ---------------------------------
    xh0 = emit_act_stats(0)
    emit_act_transposes(0, xh0)
    for g in range(NVB):
        for m in range(4 * g, 4 * g + 4):
            emit_w_tile(m)
        emit_chains(0, vbs=[g])
    xh = emit_act_stats(1)
    emit_act_transposes(1, xh)
    for c in range(1, NCHUNK):
        xh = emit_act_stats(c + 1) if c + 1 < NCHUNK else None
        emit_chains(c)
        if xh is not None:
            emit_act_transposes(c + 1, xh)


_CACHED = None


def _build():
    global _CACHED
    if _CACHED is not None:
        return _CACHED
    nc = bacc.Bacc(
        "TRN2", target_bir_lowering=False, debug=False,
        enable_asserts=False, num_devices=NCORE)
    x_d = nc.dram_tensor("x", (T, H), FP, kind="ExternalInput").ap()
    w_d = nc.dram_tensor("w", (VSH, H), FP, kind="ExternalInput").ap()
    out_d = nc.dram_tensor("out", (T, VSH), BF, kind="ExternalOutput").ap()
    with tile.TileContext(nc) as tc:
        with ExitStack() as ctx:
            _emit(ctx, tc, x_d, w_d, out_d)
    nc.compile()
    _CACHED = nc
    return nc


def kernel(hidden_states: np.ndarray, lm_weight: np.ndarray) -> np.ndarray:
    b, t, h = hidden_states.shape
    assert (b * t, h) == (T, H) and lm_weight.shape == (V, H)
    x_full = np.ascontiguousarray(
        hidden_states.reshape(T, H).astype(np.float32))
    in_maps = []
    for c in range(NCORE):
        shard = np.ascontiguousarray(
            lm_weight[c * VSH:(c + 1) * VSH].astype(np.float32))
        in_maps.append({"x": x_full, "w": shard})
    nc = _build()
    res = run_bass_kernel_spmd(nc, in_maps, core_ids=list(range(NCORE)))

    # host-side rank-1 scale: m7[chunk] * sw[v]
    xc = x_full.reshape(NCHUNK, CHUNK * H)
    m7 = (np.abs(xc).max(axis=1) * np.float32(C7)).astype(np.float32)
    m7_col = np.repeat(m7, CHUNK)[:, None]               # [T, 1]
    outs = []
    for c in range(NCORE):
        sw = np.maximum(
            np.abs(in_maps[c]["w"]).max(axis=1) * np.float32(C7),
            np.float32(1e-9)).astype(np.float32)
        arr = res.results[c]["out"].astype(np.float32)
        arr *= m7_col
        arr *= sw[None, :]
        outs.append(arr)
    full = np.concatenate(outs, axis=1)
    return full.reshape(b, t, V)


# revision 18
# speedup vs baseline: 2.1850x; 1.2820x over previous
"""Trainium2 Bass kernel for the quantized LM-head (nn_LmHeadTender), v2.

Math (per core, vocab-sharded; V shard = 4000 rows):
    Wl   = dequant_int4(lm_weight)        # per-row scale sw = rowmax/7
    y    = dequant_int4(x, per-(chunk,channel) scale s = tmax*2^(b-13)/7)
    out  = y @ Wl.T
Factor every scale out of the matmul so both operands are small integers
(times powers of two), exactly representable in fp8 e5m2:
    qw  in [-7, 7]                  (weight ints; |w/s| <= 7 so no clip)
    yq  = qx * 2^(bucket-13)        (activation ints scaled by power of 2)
    out[t, v] = (tmax_c/7) * sw[v] * sum_h yq[t, h] * qw[v, h]
The fp8 DoubleRow matmul (2 k-tiles per pass) runs at 2x bf16 rate with
exact products accumulated in fp32 PSUM.  The rank-1 output scale
m7[chunk] * sw[v] is applied on the host (exact same f32 values the
device divided by), so the device ships raw bf16 logits.

Pipeline per core:
  - weights: DMA [v,h] f32 -> DVE rowmax+quant -> ACT round -> fp8 ->
    PE transpose -> qw resident in SBUF as [h, v] fp8 (125 KB/partition)
  - acts: DMA [t,h] f32 -> gpsimd partition_all_reduce(absmax) for
    per-channel maxes (bounced via DRAM into packed [h%128, h//128]
    layout) -> bucket/scale stats on DVE -> PE transpose x (f32) ->
    ACT drains PSUM with fused  x*r + magic  (per-partition scale) ->
    DVE/ACT  (q-magic)*2^(b-13) -> y fp8 [h, t]
  - matmul: per chunk, 2x8 chains of 16 DoubleRow matmuls vs resident
    qw -> PSUM f32 -> DVE/ACT copy to bf16 -> DMA out.
Weight v-blocks are interleaved with chunk-0 chains so the PE starts
matmuls as soon as the first vocab block of qw is ready.
"""

import numpy as np

from contextlib import ExitStack

import concourse.bass as bass
import concourse.tile as tile
from concourse import bacc, bass_isa, masks, mybir
from concourse.bass_utils import run_bass_kernel_spmd

FP = mybir.dt.float32
BF = mybir.dt.bfloat16
F8 = mybir.dt.float8e5
I32 = mybir.dt.int32
ALU = mybir.AluOpType
AX = mybir.AxisListType
ACTF = mybir.ActivationFunctionType
DR = mybir.MatmulPerfMode.DoubleRow
RED = bass_isa.ReduceOp

T = 4096            # tokens (2*2048)
H = 4096            # hidden
V = 32000           # vocab
NCORE = 8
VSH = V // NCORE    # 4000 vocab rows per core
CHUNK = 256
NCHUNK = T // CHUNK         # 16
KT = H // 128               # 32 k-tiles
NKP = KT // 2               # 16 k-pairs per chain
HH = H // 2                 # 2048 h-half
DECOMP = 14
QMAX = 7.0
C_MAGIC = 12582912.0        # 1.5 * 2^23: round-to-nearest-even via add/sub
C7 = float(np.float32(1.0) / np.float32(7.0))

WT = 32                     # weight v-tiles (last one has 32 rows)
WT_ROWS = [128] * 31 + [VSH - 31 * 128]
VBS = [512] * 7 + [VSH - 7 * 512]   # v-blocks per chain group (last 416)
NVB = len(VBS)


def _emit(ctx: ExitStack, tc: "tile.TileContext", x_d, w_d, thr_d, out_d):
    nc = tc.nc

    cpool = ctx.enter_context(tc.tile_pool(name="consts", bufs=1))
    ident = cpool.tile([128, 128], FP)
    masks.make_identity(nc, ident[:])
    ident8 = cpool.tile([128, 128], F8)
    nc.vector.tensor_copy(ident8[:], ident[:])
    qw_sb = cpool.tile([128, KT, VSH], F8)   # resident quantized weight [h, v]

    thr_sb = cpool.tile([128, NCHUNK, 14], FP)  # host thresholds + tmax
    nc.sync.dma_start(thr_sb[:], thr_d[:, :].rearrange("p (c l) -> p c l", l=14))

    # persistent pools
    wpool = ctx.enter_context(tc.tile_pool(name="wstg", bufs=2))
    w8pool = ctx.enter_context(tc.tile_pool(name="w8", bufs=1))
    wsml = ctx.enter_context(tc.tile_pool(name="wsml", bufs=2))
    xpool = ctx.enter_context(tc.tile_pool(name="xin", bufs=6))
    m2pool = ctx.enter_context(tc.tile_pool(name="m2", bufs=1))
    spool = ctx.enter_context(tc.tile_pool(name="stats", bufs=1))
    q1pool = ctx.enter_context(tc.tile_pool(name="q1", bufs=2))
    ypool = ctx.enter_context(tc.tile_pool(name="y8", bufs=2))
    opool = ctx.enter_context(tc.tile_pool(name="ostg", bufs=2))

    mmps = ctx.enter_context(tc.tile_pool(name="mmps", bufs=2, space="PSUM"))
    xps = ctx.enter_context(tc.tile_pool(name="xps", bufs=4, space="PSUM"))
    m2ps = ctx.enter_context(tc.tile_pool(name="m2ps", bufs=1, space="PSUM"))
    wps = ctx.enter_context(tc.tile_pool(name="wps", bufs=1, space="PSUM"))

    y8_of = {}
    stats_of = {}

    # ---------------- weight tile m: quantize + transpose into qw_sb ----
    def emit_w_tile(m):
        rows = WT_ROWS[m]
        v0 = m * 128
        wn = []
        for hh in range(2):
            t_ = wpool.tile([128, HH], FP, tag=f"wn{hh}", name=f"wn{hh}", bufs=(2 if hh == 0 else 1))
            nc.sync.dma_start(
                t_[:rows, :], w_d[v0:v0 + rows, hh * HH:(hh + 1) * HH])
            wn.append(t_)
        r1 = wsml.tile([128, 2], FP, tag="r1", name="r1")
        for hh in range(2):
            nc.vector.tensor_reduce(
                r1[:rows, hh:hh + 1], wn[hh][:rows, :], axis=AX.X, op=ALU.max,
                apply_absolute_value=True)
        rmax = wsml.tile([128, 1], FP, tag="rmax", name="rmax")
        nc.vector.tensor_reduce(
            rmax[:rows, :], r1[:rows, :], axis=AX.X, op=ALU.max)
        sw = wsml.tile([128, 1], FP, tag="sw", name="sw")
        nc.vector.tensor_scalar(
            sw[:rows, :], rmax[:rows, :], C7, 1e-9, ALU.mult, ALU.max)
        rw = wsml.tile([128, 1], FP, tag="rw", name="rw")
        nc.vector.reciprocal(rw[:rows, :], sw[:rows, :])
        w8 = []
        for hh in range(2):
            # q + magic  (round-to-nearest-even)
            nc.vector.tensor_scalar(
                wn[hh][:rows, :], wn[hh][:rows, :], rw[:rows, :], C_MAGIC,
                ALU.mult, ALU.add)
            q8 = w8pool.tile([128, HH], F8, tag=f"w8{hh}", name=f"q8{hh}")
            nc.scalar.activation(
                q8[:rows, :], wn[hh][:rows, :], ACTF.Copy, bias=-C_MAGIC,
                scale=1.0)
            w8.append(q8)
        # transpose 32 k-blocks into qw_sb[:, k, v0:v0+rows]
        # (fp8 transpose writes PSUM with element step 2 -> interleaved tile)
        for q in range(4):
            ps = wps.tile([128, 8, 128, 2], F8, tag="wps", name="wps")
            for j in range(8):
                k = q * 8 + j
                src = w8[k // 16][:rows, (k % 16) * 128:(k % 16 + 1) * 128]
                nc.tensor.transpose(
                    ps[:, j, :rows, 0], src, ident8[:rows, :rows])
            nc.scalar.copy(
                qw_sb[:, q * 8:(q + 1) * 8, v0:v0 + rows],
                ps[:, :, :rows, 0])

    # ---------------- activation stats for chunk c ----------------------
    # x arrives as 8 quarter tiles [128, 1024]: (rt, q) pairs. Channel maxes
    # via DVE abs_max of the two row-tiles + PE transpose + PSUM reduce.
    def emit_act_stats_a(c):
        xh = [[None] * 4, [None] * 4]
        cmax = spool.tile([128, KT], FP, tag="cmax", name="cmax")
        QW = H // 4
        for q in range(4):
            for rt in range(2):
                t_ = xpool.tile([128, QW], FP, tag="x", name="x")
                nc.sync.dma_start(
                    t_[:],
                    x_d[c * CHUNK + rt * 128:c * CHUNK + (rt + 1) * 128,
                        q * QW:(q + 1) * QW])
                xh[rt][q] = t_
            for f in range(2):
                sl = slice(f * (QW // 2), (f + 1) * (QW // 2))
                m2q = m2pool.tile([128, QW // 2], FP, tag="m2", name="m2q")
                m2v = m2pool.tile([128, QW // 2], FP, tag="m2v", name="m2v")
                nc.vector.tensor_tensor(
                    m2q[:], xh[0][q][:, sl], xh[1][q][:, sl], op=ALU.max)
                nc.vector.tensor_tensor(
                    m2v[:], xh[0][q][:, sl], xh[1][q][:, sl], op=ALU.min)
                nc.vector.scalar_tensor_tensor(
                    m2q[:], m2v[:], -1.0, m2q[:], op0=ALU.mult, op1=ALU.max)
                ps = m2ps.tile([128, 4, 128], FP, tag="m2ps", name="m2ps")
                for j in range(4):
                    nc.tensor.transpose(
                        ps[:, j, :], m2q[:, j * 128:(j + 1) * 128], ident[:])
                nc.vector.tensor_reduce(
                    cmax[:, q * 8 + f * 4:q * 8 + f * 4 + 4], ps[:],
                    axis=AX.X, op=ALU.max)
        stats_of[c] = [cmax, xh]
        return xh

    def emit_act_stats_b(c):
        cmax = stats_of[c][0]
        thr_c = spool.tile([128, 14], FP, tag="thr_c", name="thr_c")
        nc.vector.tensor_copy(thr_c[:], thr_sb[:, c, :])
        bucket = spool.tile([128, KT], FP, tag="bucket", name="bucket")
        nc.vector.memset(bucket[:], 0.0)
        for lv in range(DECOMP - 1):
            nc.vector.scalar_tensor_tensor(
                bucket[:], cmax[:], thr_c[:, lv:lv + 1], bucket[:],
                op0=ALU.is_gt, op1=ALU.add)
        g = spool.tile([128, KT], FP, tag="g", name="g")
        nc.vector.tensor_scalar(
            g[:], bucket[:], 114.0, 8388608.0, ALU.add, ALU.mult)
        g_i = spool.tile([128, KT], I32, tag="g_i", name="g_i", bufs=2)
        nc.vector.tensor_copy(g_i[:], g[:])
        pw = g_i[:].bitcast(FP)
        ch_thr = spool.tile([128, KT], FP, tag="ch_thr", name="ch_thr")
        nc.vector.tensor_scalar(
            ch_thr[:], pw, thr_c[:, 13:14], None, ALU.mult)
        s_t = spool.tile([128, KT], FP, tag="s_t", name="s_t")
        nc.vector.tensor_scalar(
            s_t[:], ch_thr[:], C7, 1e-9, ALU.mult, ALU.max)
        r_t = spool.tile([128, KT], FP, tag="r_t", name="r_t", bufs=2)
        nc.vector.reciprocal(r_t[:], s_t[:])
        bias_k = spool.tile([128, KT], FP, tag="bias_k", name="bias_k", bufs=2)
        nc.vector.tensor_scalar(bias_k[:], pw, -C_MAGIC, None, ALU.mult)
        stats_of[c] = [cmax, stats_of[c][1], g_i, r_t, bias_k]

    # ---------------- transpose + fused quant (per h-quarter) -----------
    def emit_act_transposes(c, qs):
        _, xh, g_i, r_t, bias_k = stats_of[c]
        if qs[0] == 0:
            y8_of[c] = ypool.tile([128, KT, CHUNK], F8, tag="y8", name="y8")
        y8 = y8_of[c]
        for q in qs:
            for kk in range(4):        # k-pairs within quarter
                ps = xps.tile([128, 2, 2, 128], FP, tag="xps", name="ps")
                for ki in range(2):
                    k = q * 8 + kk * 2 + ki
                    for tb in range(2):
                        nc.tensor.transpose(
                            ps[:, ki, tb, :],
                            xh[tb][q][:, (k % 8) * 128:(k % 8 + 1) * 128],
                            ident[:])
                for ki in range(2):
                    k = q * 8 + kk * 2 + ki
                    q1 = q1pool.tile([128, 2, 128], FP, tag="q1", name="q1")
                    nc.scalar.activation(
                        q1[:], ps[:, ki, :, :], ACTF.Copy, bias=C_MAGIC,
                        scale=r_t[:, k:k + 1])
                    dst = y8[:, k, :]
                    q1f = q1[:].rearrange("p a b -> p (a b)")
                    pw_col = g_i[:, k:k + 1].bitcast(FP)
                    if k % 2 == 0:
                        nc.vector.tensor_scalar(
                            dst, q1f, C_MAGIC, pw_col,
                            ALU.subtract, ALU.mult)
                    else:
                        nc.scalar.activation(
                            dst, q1f, ACTF.Identity,
                            bias=bias_k[:, k:k + 1], scale=pw_col)

    # ---------------- matmul chains for chunk c -------------------------
    def emit_chains(c, vbs=None):
        y8 = y8_of[c]
        for tb in range(2):
            for vb in (range(NVB) if vbs is None else vbs):
                wv = VBS[vb]
                ps = mmps.tile([128, 512], FP, tag="mm", name="mmps")
                for kp in range(NKP):
                    nc.tensor.matmul(
                        ps[:, :wv],
                        y8[:, 2 * kp:2 * kp + 2, tb * 128:(tb + 1) * 128],
                        qw_sb[:, 2 * kp:2 * kp + 2, vb * 512:vb * 512 + wv],
                        start=(kp == 0), stop=(kp == NKP - 1), perf_mode=DR)
                stg = opool.tile([128, 512], BF, tag="stg", name="stg")
                if vb % 2 == 0:
                    nc.vector.tensor_copy(stg[:, :wv], ps[:, :wv])
                else:
                    nc.scalar.copy(stg[:, :wv], ps[:, :wv])
                nc.sync.dma_start(
                    out_d[c * CHUNK + tb * 128:c * CHUNK + (tb + 1) * 128,
                          vb * 512:vb * 512 + wv],
                    stg[:, :wv])

    # ---------------- emission schedule ---------------------------------
    def emit_stats_and_q0(c):
        emit_act_stats_a(c)
        emit_act_stats_b(c)
        emit_act_transposes(c, [0])

    emit_stats_and_q0(0)
    emit_act_transposes(0, [1, 2, 3])
    for g in range(NVB):
        for m in range(4 * g, 4 * g + 4):
            emit_w_tile(m)
        emit_chains(0, vbs=[g])
    emit_stats_and_q0(1)
    emit_act_transposes(1, [1, 2, 3])
    for c in range(1, NCHUNK):
        if c + 1 < NCHUNK:
            emit_stats_and_q0(c + 1)
        emit_chains(c)
        if c + 1 < NCHUNK:
            emit_act_transposes(c + 1, [1, 2, 3])

_CACHED = None


def _build():
    global _CACHED
    if _CACHED is not None:
        return _CACHED
    nc = bacc.Bacc(
        "TRN2", target_bir_lowering=False, debug=False,
        enable_asserts=False, num_devices=NCORE)
    x_d = nc.dram_tensor("x", (T, H), FP, kind="ExternalInput").ap()
    w_d = nc.dram_tensor("w", (VSH, H), FP, kind="ExternalInput").ap()
    thr_d = nc.dram_tensor("thr", (128, NCHUNK * 14), FP,
                           kind="ExternalInput").ap()
    out_d = nc.dram_tensor("out", (T, VSH), BF, kind="ExternalOutput").ap()
    with tile.TileContext(nc) as tc:
        with ExitStack() as ctx:
            _emit(ctx, tc, x_d, w_d, thr_d, out_d)
    nc.compile()
    _CACHED = nc
    return nc


def kernel(hidden_states: np.ndarray, lm_weight: np.ndarray) -> np.ndarray:
    b, t, h = hidden_states.shape
    assert (b * t, h) == (T, H) and lm_weight.shape == (V, H)
    x_full = np.ascontiguousarray(
        hidden_states.reshape(T, H).astype(np.float32))
    xc = x_full.reshape(NCHUNK, CHUNK * H)
    tmax = np.abs(xc).max(axis=1).astype(np.float32)         # [NCHUNK]
    lv = np.arange(14, dtype=np.float32)
    thr = tmax[:, None] * np.exp2(lv - 13.0)[None, :].astype(np.float32)
    thr[:, 13] = tmax
    thr_np = np.ascontiguousarray(
        np.broadcast_to(thr.reshape(1, NCHUNK * 14).astype(np.float32),
                        (128, NCHUNK * 14)))
    in_maps = []
    for c in range(NCORE):
        shard = np.ascontiguousarray(
            lm_weight[c * VSH:(c + 1) * VSH].astype(np.float32))
        in_maps.append({"x": x_full, "w": shard, "thr": thr_np})
    nc = _build()
    res = run_bass_kernel_spmd(nc, in_maps, core_ids=list(range(NCORE)))

    # host-side rank-1 scale: m7[chunk] * sw[v]
    m7 = (tmax * np.float32(C7)).astype(np.float32)
    m7_col = np.repeat(m7, CHUNK)[:, None]               # [T, 1]
    outs = []
    for c in range(NCORE):
        sw = np.maximum(
            np.abs(in_maps[c]["w"]).max(axis=1) * np.float32(C7),
            np.float32(1e-9)).astype(np.float32)
        arr = res.results[c]["out"].astype(np.float32)
        arr *= m7_col
        arr *= sw[None, :]
        outs.append(arr)
    full = np.concatenate(outs, axis=1)
    return full.reshape(b, t, V)


# revision 19
# speedup vs baseline: 2.2829x; 1.0448x over previous
"""Trainium2 Bass kernel for the quantized LM-head (nn_LmHeadTender), v2.

Math (per core, vocab-sharded; V shard = 4000 rows):
    Wl   = dequant_int4(lm_weight)        # per-row scale sw = rowmax/7
    y    = dequant_int4(x, per-(chunk,channel) scale s = tmax*2^(b-13)/7)
    out  = y @ Wl.T
Factor every scale out of the matmul so both operands are small integers
(times powers of two), exactly representable in fp8 e5m2:
    qw  in [-7, 7]                  (weight ints; |w/s| <= 7 so no clip)
    yq  = qx * 2^(bucket-13)        (activation ints scaled by power of 2)
    out[t, v] = (tmax_c/7) * sw[v] * sum_h yq[t, h] * qw[v, h]
The fp8 DoubleRow matmul (2 k-tiles per pass) runs at 2x bf16 rate with
exact products accumulated in fp32 PSUM.  The rank-1 output scale
m7[chunk] * sw[v] is applied on the host (exact same f32 values the
device divided by), so the device ships raw bf16 logits.

Pipeline per core:
  - weights: DMA [v,h] f32 -> DVE rowmax+quant -> ACT round -> fp8 ->
    PE transpose -> qw resident in SBUF as [h, v] fp8 (125 KB/partition)
  - acts: DMA [t,h] f32 -> gpsimd partition_all_reduce(absmax) for
    per-channel maxes (bounced via DRAM into packed [h%128, h//128]
    layout) -> bucket/scale stats on DVE -> PE transpose x (f32) ->
    ACT drains PSUM with fused  x*r + magic  (per-partition scale) ->
    DVE/ACT  (q-magic)*2^(b-13) -> y fp8 [h, t]
  - matmul: per chunk, 2x8 chains of 16 DoubleRow matmuls vs resident
    qw -> PSUM f32 -> DVE/ACT copy to bf16 -> DMA out.
Weight v-blocks are interleaved with chunk-0 chains so the PE starts
matmuls as soon as the first vocab block of qw is ready.
"""

import numpy as np

from contextlib import ExitStack

import concourse.bass as bass
import concourse.tile as tile
from concourse import bacc, bass_isa, masks, mybir
from concourse.bass_utils import run_bass_kernel_spmd

FP = mybir.dt.float32
BF = mybir.dt.bfloat16
F8 = mybir.dt.float8e5
I32 = mybir.dt.int32
ALU = mybir.AluOpType
AX = mybir.AxisListType
ACTF = mybir.ActivationFunctionType
DR = mybir.MatmulPerfMode.DoubleRow
RED = bass_isa.ReduceOp

T = 4096            # tokens (2*2048)
H = 4096            # hidden
V = 32000           # vocab
NCORE = 8
VSH = V // NCORE    # 4000 vocab rows per core
CHUNK = 256
NCHUNK = T // CHUNK         # 16
KT = H // 128               # 32 k-tiles
NKP = KT // 2               # 16 k-pairs per chain
HH = H // 2                 # 2048 h-half
DECOMP = 14
QMAX = 7.0
C_MAGIC = 12582912.0        # 1.5 * 2^23: round-to-nearest-even via add/sub
C7 = float(np.float32(1.0) / np.float32(7.0))

WT = 32                     # weight v-tiles (last one has 32 rows)
WT_ROWS = [128] * 31 + [VSH - 31 * 128]
VBS = [512] * 7 + [VSH - 7 * 512]   # v-blocks per chain group (last 416)
NVB = len(VBS)


def _emit(ctx: ExitStack, tc: "tile.TileContext", x_d, w_d, thr_d, out_d):
    nc = tc.nc

    cpool = ctx.enter_context(tc.tile_pool(name="consts", bufs=1))
    ident = cpool.tile([128, 128], FP)
    masks.make_identity(nc, ident[:])
    ident8 = cpool.tile([128, 128], F8)
    nc.vector.tensor_copy(ident8[:], ident[:])
    qw_sb = cpool.tile([128, KT, VSH], F8)   # resident quantized weight [h, v]

    thr_sb = cpool.tile([128, NCHUNK, 14], FP)  # host thresholds + tmax
    nc.sync.dma_start(thr_sb[:], thr_d[:, :].rearrange("p (c l) -> p c l", l=14))

    # persistent pools
    wpool = ctx.enter_context(tc.tile_pool(name="wstg", bufs=2))
    w8pool = ctx.enter_context(tc.tile_pool(name="w8", bufs=1))
    wsml = ctx.enter_context(tc.tile_pool(name="wsml", bufs=2))
    xpool = ctx.enter_context(tc.tile_pool(name="xin", bufs=6))
    m2pool = ctx.enter_context(tc.tile_pool(name="m2", bufs=1))
    spool = ctx.enter_context(tc.tile_pool(name="stats", bufs=1))
    q1pool = ctx.enter_context(tc.tile_pool(name="q1", bufs=2))
    ypool = ctx.enter_context(tc.tile_pool(name="y8", bufs=2))
    opool = ctx.enter_context(tc.tile_pool(name="ostg", bufs=2))

    mmps = ctx.enter_context(tc.tile_pool(name="mmps", bufs=2, space="PSUM"))
    xps = ctx.enter_context(tc.tile_pool(name="xps", bufs=4, space="PSUM"))
    m2ps = ctx.enter_context(tc.tile_pool(name="m2ps", bufs=1, space="PSUM"))
    wps = ctx.enter_context(tc.tile_pool(name="wps", bufs=1, space="PSUM"))

    y8_of = {}
    stats_of = {}

    # ---------------- weight tile m: quantize + transpose into qw_sb ----
    def emit_w_tile(m):
        rows = WT_ROWS[m]
        v0 = m * 128
        wn = []
        for hh in range(2):
            t_ = wpool.tile([128, HH], FP, tag=f"wn{hh}", name=f"wn{hh}", bufs=(2 if hh == 0 else 1))
            nc.sync.dma_start(
                t_[:rows, :], w_d[v0:v0 + rows, hh * HH:(hh + 1) * HH])
            wn.append(t_)
        r1 = wsml.tile([128, 2], FP, tag="r1", name="r1")
        for hh in range(2):
            nc.vector.tensor_reduce(
                r1[:rows, hh:hh + 1], wn[hh][:rows, :], axis=AX.X, op=ALU.max,
                apply_absolute_value=True)
        rmax = wsml.tile([128, 1], FP, tag="rmax", name="rmax")
        nc.vector.tensor_reduce(
            rmax[:rows, :], r1[:rows, :], axis=AX.X, op=ALU.max)
        sw = wsml.tile([128, 1], FP, tag="sw", name="sw")
        nc.vector.tensor_scalar(
            sw[:rows, :], rmax[:rows, :], C7, 1e-9, ALU.mult, ALU.max)
        rw = wsml.tile([128, 1], FP, tag="rw", name="rw")
        nc.vector.reciprocal(rw[:rows, :], sw[:rows, :])
        w8 = []
        for hh in range(2):
            # q + magic  (round-to-nearest-even)
            nc.vector.tensor_scalar(
                wn[hh][:rows, :], wn[hh][:rows, :], rw[:rows, :], C_MAGIC,
                ALU.mult, ALU.add)
            q8 = w8pool.tile([128, HH], F8, tag=f"w8{hh}", name=f"q8{hh}")
            nc.scalar.activation(
                q8[:rows, :], wn[hh][:rows, :], ACTF.Copy, bias=-C_MAGIC,
                scale=1.0)
            w8.append(q8)
        # transpose 32 k-blocks into qw_sb[:, k, v0:v0+rows]
        # (fp8 transpose writes PSUM with element step 2 -> interleaved tile)
        for q in range(4):
            ps = wps.tile([128, 8, 128, 2], F8, tag="wps", name="wps")
            for j in range(8):
                k = q * 8 + j
                src = w8[k // 16][:rows, (k % 16) * 128:(k % 16 + 1) * 128]
                nc.tensor.transpose(
                    ps[:, j, :rows, 0], src, ident8[:rows, :rows])
            nc.scalar.copy(
                qw_sb[:, q * 8:(q + 1) * 8, v0:v0 + rows],
                ps[:, :, :rows, 0])

    # ---------------- activation stats for chunk c ----------------------
    # x arrives as 8 quarter tiles [128, 1024]: (rt, q) pairs. Channel maxes
    # via DVE abs_max of the two row-tiles + PE transpose + PSUM reduce.
    def emit_act_stats_a(c):
        xh = [[None] * 4, [None] * 4]
        cmax = spool.tile([128, KT], FP, tag="cmax", name="cmax")
        QW = H // 4
        for q in range(4):
            for rt in range(2):
                t_ = xpool.tile([128, QW], FP, tag="x", name="x")
                nc.sync.dma_start(
                    t_[:],
                    x_d[c * CHUNK + rt * 128:c * CHUNK + (rt + 1) * 128,
                        q * QW:(q + 1) * QW])
                xh[rt][q] = t_
            for f in range(2):
                sl = slice(f * (QW // 2), (f + 1) * (QW // 2))
                m2q = m2pool.tile([128, QW // 2], FP, tag="m2", name="m2q")
                m2v = m2pool.tile([128, QW // 2], FP, tag="m2v", name="m2v")
                nc.vector.tensor_tensor(
                    m2q[:], xh[0][q][:, sl], xh[1][q][:, sl], op=ALU.max)
                nc.vector.tensor_tensor(
                    m2v[:], xh[0][q][:, sl], xh[1][q][:, sl], op=ALU.min)
                nc.vector.scalar_tensor_tensor(
                    m2q[:], m2v[:], -1.0, m2q[:], op0=ALU.mult, op1=ALU.max)
                ps = m2ps.tile([128, 4, 128], FP, tag="m2ps", name="m2ps")
                for j in range(4):
                    nc.tensor.transpose(
                        ps[:, j, :], m2q[:, j * 128:(j + 1) * 128], ident[:])
                nc.vector.tensor_reduce(
                    cmax[:, q * 8 + f * 4:q * 8 + f * 4 + 4], ps[:],
                    axis=AX.X, op=ALU.max)
        stats_of[c] = [cmax, xh]
        return xh

    def emit_act_stats_b(c):
        cmax = stats_of[c][0]
        thr_c = spool.tile([128, 14], FP, tag="thr_c", name="thr_c")
        nc.vector.tensor_copy(thr_c[:], thr_sb[:, c, :])
        bucket = spool.tile([128, KT], FP, tag="bucket", name="bucket")
        nc.vector.memset(bucket[:], 0.0)
        for lv in range(DECOMP - 1):
            nc.vector.scalar_tensor_tensor(
                bucket[:], cmax[:], thr_c[:, lv:lv + 1], bucket[:],
                op0=ALU.is_gt, op1=ALU.add)
        g = spool.tile([128, KT], FP, tag="g", name="g")
        nc.vector.tensor_scalar(
            g[:], bucket[:], 114.0, 8388608.0, ALU.add, ALU.mult)
        g_i = spool.tile([128, KT], I32, tag="g_i", name="g_i", bufs=2)
        nc.vector.tensor_copy(g_i[:], g[:])
        pw = g_i[:].bitcast(FP)
        ch_thr = spool.tile([128, KT], FP, tag="ch_thr", name="ch_thr")
        nc.vector.tensor_scalar(
            ch_thr[:], pw, thr_c[:, 13:14], None, ALU.mult)
        s_t = spool.tile([128, KT], FP, tag="s_t", name="s_t")
        nc.vector.tensor_scalar(
            s_t[:], ch_thr[:], C7, 1e-9, ALU.mult, ALU.max)
        r_t = spool.tile([128, KT], FP, tag="r_t", name="r_t", bufs=2)
        nc.vector.reciprocal(r_t[:], s_t[:])
        stats_of[c] = [cmax, stats_of[c][1], g_i, r_t, None]

    # ---------------- transpose + fused quant (per h-quarter) -----------
    def emit_act_transposes(c, qs):
        _, xh, g_i, r_t, bias_k = stats_of[c]
        if qs[0] == 0:
            y8_of[c] = ypool.tile([128, KT, CHUNK], F8, tag="y8", name="y8")
        y8 = y8_of[c]
        for q in qs:
            for kk in range(4):        # k-pairs within quarter
                ps = xps.tile([128, 2, 2, 128], FP, tag="xps", name="ps")
                for ki in range(2):
                    k = q * 8 + kk * 2 + ki
                    for tb in range(2):
                        nc.tensor.transpose(
                            ps[:, ki, tb, :],
                            xh[tb][q][:, (k % 8) * 128:(k % 8 + 1) * 128],
                            ident[:])
                for ki in range(2):
                    k = q * 8 + kk * 2 + ki
                    q1 = q1pool.tile([128, 2, 128], FP, tag="q1", name="q1")
                    nc.scalar.activation(
                        q1[:], ps[:, ki, :, :], ACTF.Copy, bias=C_MAGIC,
                        scale=r_t[:, k:k + 1])
                    dst = y8[:, k, :]
                    q1f = q1[:].rearrange("p a b -> p (a b)")
                    pw_col = g_i[:, k:k + 1].bitcast(FP)
                    nc.vector.tensor_scalar(
                        dst, q1f, C_MAGIC, pw_col,
                        ALU.subtract, ALU.mult)

    # ---------------- matmul chains for chunk c -------------------------
    def emit_chains(c, vbs=None):
        y8 = y8_of[c]
        for tb in range(2):
            for vb in (range(NVB) if vbs is None else vbs):
                wv = VBS[vb]
                ps = mmps.tile([128, 512], FP, tag="mm", name="mmps")
                for kp in range(NKP):
                    nc.tensor.matmul(
                        ps[:, :wv],
                        y8[:, 2 * kp:2 * kp + 2, tb * 128:(tb + 1) * 128],
                        qw_sb[:, 2 * kp:2 * kp + 2, vb * 512:vb * 512 + wv],
                        start=(kp == 0), stop=(kp == NKP - 1), perf_mode=DR)
                stg = opool.tile([128, 512], BF, tag="stg", name="stg")
                nc.scalar.copy(stg[:, :wv], ps[:, :wv])
                nc.sync.dma_start(
                    out_d[c * CHUNK + tb * 128:c * CHUNK + (tb + 1) * 128,
                          vb * 512:vb * 512 + wv],
                    stg[:, :wv])

    # ---------------- emission schedule ---------------------------------
    def emit_stats_and_q0(c):
        emit_act_stats_a(c)
        emit_act_stats_b(c)
        emit_act_transposes(c, [0])

    emit_stats_and_q0(0)
    emit_act_transposes(0, [1, 2, 3])
    emit_stats_and_q0(1)
    emit_act_transposes(1, [1, 2, 3])
    for g in range(NVB):
        for m in range(4 * g, 4 * g + 4):
            emit_w_tile(m)
        emit_chains(0, vbs=[g])
        emit_chains(1, vbs=[g])
    emit_stats_and_q0(2)
    emit_act_transposes(2, [1, 2, 3])
    for c in range(2, NCHUNK):
        if c + 1 < NCHUNK:
            emit_stats_and_q0(c + 1)
        emit_chains(c)
        if c + 1 < NCHUNK:
            emit_act_transposes(c + 1, [1, 2, 3])

_CACHED = None


def _build():
    global _CACHED
    if _CACHED is not None:
        return _CACHED
    nc = bacc.Bacc(
        "TRN2", target_bir_lowering=False, debug=False,
        enable_asserts=False, num_devices=NCORE)
    x_d = nc.dram_tensor("x", (T, H), FP, kind="ExternalInput").ap()
    w_d = nc.dram_tensor("w", (VSH, H), FP, kind="ExternalInput").ap()
    thr_d = nc.dram_tensor("thr", (128, NCHUNK * 14), FP,
                           kind="ExternalInput").ap()
    out_d = nc.dram_tensor("out", (T, VSH), BF, kind="ExternalOutput").ap()
    with tile.TileContext(nc) as tc:
        with ExitStack() as ctx:
            _emit(ctx, tc, x_d, w_d, thr_d, out_d)
    nc.compile()
    _CACHED = nc
    return nc


def kernel(hidden_states: np.ndarray, lm_weight: np.ndarray) -> np.ndarray:
    b, t, h = hidden_states.shape
    assert (b * t, h) == (T, H) and lm_weight.shape == (V, H)
    x_full = np.ascontiguousarray(
        hidden_states.reshape(T, H).astype(np.float32))
    xc = x_full.reshape(NCHUNK, CHUNK * H)
    tmax = np.abs(xc).max(axis=1).astype(np.float32)         # [NCHUNK]
    lv = np.arange(14, dtype=np.float32)
    thr = tmax[:, None] * np.exp2(lv - 13.0)[None, :].astype(np.float32)
    thr[:, 13] = tmax
    thr_np = np.ascontiguousarray(
        np.broadcast_to(thr.reshape(1, NCHUNK * 14).astype(np.float32),
                        (128, NCHUNK * 14)))
    in_maps = []
    for c in range(NCORE):
        shard = np.ascontiguousarray(
            lm_weight[c * VSH:(c + 1) * VSH].astype(np.float32))
        in_maps.append({"x": x_full, "w": shard, "thr": thr_np})
    nc = _build()
    res = run_bass_kernel_spmd(nc, in_maps, core_ids=list(range(NCORE)))

    # host-side rank-1 scale: m7[chunk] * sw[v]
    m7 = (tmax * np.float32(C7)).astype(np.float32)
    m7_col = np.repeat(m7, CHUNK)[:, None]               # [T, 1]
    outs = []
    for c in range(NCORE):
        sw = np.maximum(
            np.abs(in_maps[c]["w"]).max(axis=1) * np.float32(C7),
            np.float32(1e-9)).astype(np.float32)
        arr = res.results[c]["out"].astype(np.float32)
        arr *= m7_col
        arr *= sw[None, :]
        outs.append(arr)
    full = np.concatenate(outs, axis=1)
    return full.reshape(b, t, V)
